# revision 24
# baseline (speedup 1.0000x reference)
"""Trainium2 Bass kernel for nn_CrossAttention_249108103802.

Math (per batch b, one NeuronCore; 8 cores data-parallel over B=8):
  q_s   = heads(x_s)                   (column slices of x_s)
  k,v   = x_s @ W_s  split per head    -> never materialized; instead
  ctx_s = softmax_d(scale * k^T v)     via the Gram trick:
          k_h^T v_h = Wk_h^T (x^T x) Wv_h
  o1    = q1 @ blockdiag(ctx2), o2 = q2 @ blockdiag(ctx1)

Precision: x and W are cast to bf16 on the host; all PE matmuls run in
bf16 with fp32 PSUM accumulation. The Gram matrix is split G = Gc + mu*I
(mu = N) so Gc fits bf16; the exact mu * Wv^T Wk correction is computed
on the host in fp64 and shipped as an fp32 input. Softmax subtracts the
per-row max before exp (the logits reach ~92, which overflows fp32 exp).
Measured end-to-end rel err ~4e-3 vs the fp32 reference.
"""
import sys

sys.path.insert(0, "/opt/trn_rl_repo")

import ml_dtypes
import numpy as np

import concourse.bass as bass
import concourse.mybir as mybir
import concourse.tile as tile
from concourse import bacc
from concourse.bass_utils import run_bass_kernel_spmd
from concourse.masks import make_identity

B, N, C, H = 8, 4096, 512, 8
HD = C // H                    # 64
SCALE = HD ** -0.5             # 1/8
MU = float(N)                  # expected Gram diagonal
NT = N // 128                  # 32 row tiles
CB = C // 128                  # 4 feature blocks
NCH = NT // 4                  # 8 chunks of 4 row tiles
BF = mybir.dt.bfloat16
F32 = mybir.dt.float32
AF = mybir.ActivationFunctionType


def build():
    nc = bacc.Bacc("TRN2", target_bir_lowering=False, debug=False, num_devices=8)
    x_d = [nc.declare_dram_parameter("x1", [N, C], BF, isOutput=False),
           nc.declare_dram_parameter("x2", [N, C], BF, isOutput=False)]
    w_d = [nc.declare_dram_parameter("w1", [C, 2 * C], BF, isOutput=False),
           nc.declare_dram_parameter("w2", [C, 2 * C], BF, isOutput=False)]
    t_d = [nc.declare_dram_parameter("tt1", [HD, C], F32, isOutput=False),
           nc.declare_dram_parameter("tt2", [HD, C], F32, isOutput=False)]
    o_d = [nc.declare_dram_parameter("o1", [N, C], BF, isOutput=True),
           nc.declare_dram_parameter("o2", [N, C], BF, isOutput=True)]

    with tile.TileContext(nc) as tc:
        with (
            tc.tile_pool(name="const", bufs=1) as constp,
            tc.tile_pool(name="w", bufs=1) as wp,
            tc.tile_pool(name="x0", bufs=4) as xp0,
            tc.tile_pool(name="x1", bufs=4) as xp1,
            tc.tile_pool(name="xt", bufs=1) as xtp,
            tc.tile_pool(name="g", bufs=1) as gp_,
            tc.tile_pool(name="a", bufs=1) as ap_,
            tc.tile_pool(name="ctx", bufs=1) as cxp,
            tc.tile_pool(name="osb", bufs=3) as osp,
            tc.tile_pool(name="ps_g", bufs=1, space="PSUM") as psg,
            tc.tile_pool(name="ps_t", bufs=2, space="PSUM") as pst,
            tc.tile_pool(name="ps_a", bufs=1, space="PSUM") as psa,
        ):
            ident = constp.tile([128, 128], BF, tag="ident")
            make_identity(nc, ident[:])
            muI = constp.tile([128, 128], F32, tag="muI")
            nc.gpsimd.memset(muI[:], 0.0)
            nc.gpsimd.affine_select(
                out=muI[:], in_=muI[:],
                compare_op=mybir.AluOpType.not_equal, fill=MU,
                base=0, pattern=[[-1, 128]], channel_multiplier=1,
            )

            def copy_alt(i, out, in_):
                if i % 2 == 0:
                    nc.scalar.copy(out, in_)
                else:
                    nc.vector.tensor_copy(out, in_)

            xts, cbds = [], []
            for s in range(2):
                xts.append(xtp.tile([128, CB, N], BF, tag=f"xt{s}",
                                    name=f"xt{s}"))
                cbds.append([cxp.tile([128, 128], BF, tag=f"cbd{s}_{cb}",
                                      name=f"cbd{s}_{cb}") for cb in range(CB)])

            def load_chunk(s, r):
                """DMA one [512, C] chunk of x_s as bf16 into SBUF."""
                xp = xp0 if s == 0 else xp1
                xc = xp.tile([128, 4, C], BF, tag=f"xc{s}", name=f"xc{s}_{r}")
                src = x_d[s][512 * r:512 * (r + 1), :].rearrange(
                    "(t p) c -> p t c", p=128)
                nc.sync.dma_start(out=xc[:], in_=src)
                return xc

            tp8s = {}

            def gram_tile(s, gps, xc, tt_, t):
                """Gram accum + transpose of one [128, C] row tile.
                Transposes of tile pairs (2t, 2t+1) land in one PSUM bank,
                laid out [cb][t%2][col], and are evacuated with one copy."""
                for m in range(CB):
                    nc.tensor.matmul(
                        gps[m],
                        lhsT=xc[:, tt_, 128 * m:128 * (m + 1)],
                        rhs=xc[:, tt_, 128 * m:],
                        start=(t == 0), stop=(t == NT - 1),
                    )
                if t % 2 == 0:
                    tp8s[s] = pst.tile([128, CB, 2, 128], BF, tag="tps",
                                       name=f"tp8_{s}_{t}")
                tp8 = tp8s[s]
                for cb in range(CB):
                    nc.tensor.transpose(
                        tp8[:, cb, t % 2, :], xc[:, tt_, 128 * cb:128 * (cb + 1)],
                        ident[:])
                if t % 2 == 1:
                    copy_alt(t // 2, xts[s][:, :, 128 * (t - 1):128 * (t + 1)],
                             tp8[:])

            obs = {}

            def out_tile(s, t, gp_tags=False):
                """One [128, C] row tile of o_s = x_s @ blockdiag(ctx_other):
                matmul into a rotating PSUM slot, then copy into the chunk's
                output staging tile; DMA the chunk once its 4 tiles landed.
                gp_tags=True cycles through the idle Gram banks (4 slots)
                instead of the 2 ctx banks."""
                r, tt_ = t // 4, t % 4
                if tt_ == 0:
                    obs[(s, r)] = osp.tile([128, 4, C], BF, tag="ob",
                                           name=f"ob{s}_{r}")
                if gp_tags:
                    cyc = ("gp0", "gp1", "gp2", "gp3", "apx", "ctp")[t % 6]
                    pool = psg if cyc.startswith("gp") else psa
                    op = pool.tile([128, C], F32, tag=cyc, name=f"op{s}_{t}")
                else:
                    op = psa.tile([128, C], F32, tag=("apx", "ctp")[t % 2],
                                  name=f"op{s}_{t}")
                for cb in range(CB):
                    nc.tensor.matmul(
                        op[:, 128 * cb:128 * (cb + 1)],
                        lhsT=xts[s][:, cb, 128 * t:128 * (t + 1)],
                        rhs=cbds[1 - s][cb][:, :],
                        start=True, stop=True)
                ob = obs[(s, r)]
                copy_alt(t, ob[:, tt_, :], op[:])
                if tt_ == 3:
                    dst = o_d[s][512 * r:512 * (r + 1), :].rearrange(
                        "(t p) c -> p t c", p=128)
                    nc.sync.dma_start(out=dst, in_=ob[:])

            def ctx_a(s, gps):
                """PE-heavy first half: G_s -> Gc -> A -> ctp logits (PSUM)."""
                # Gc = G - mu*I in bf16; gsb[:, a, f] = G[128a+p, f]
                gsb = gp_.tile([128, CB, C], BF, tag="gsb", name=f"g{s}")
                for m in range(CB):
                    nc.vector.tensor_sub(
                        gsb[:, m, 128 * m:128 * (m + 1)],
                        gps[m][:, 0:128], muI[:])
                    if m < CB - 1:
                        copy_alt(m, gsb[:, m, 128 * (m + 1):], gps[m][:, 128:])
                # lower-triangle tiles by PE transpose
                low = {}
                gtr = gp_.tile([128, 6, 128], BF, tag="gtr", name=f"gt{s}")
                idx = 0
                for a2 in range(CB):
                    for b2 in range(a2):
                        tp = pst.tile([128, 128], BF, tag="tps",
                                      name=f"gtp{s}_{idx}")
                        nc.tensor.transpose(
                            tp[:], gsb[:, b2, 128 * a2:128 * (a2 + 1)], ident[:])
                        copy_alt(idx, gtr[:, idx, :], tp[:])
                        low[(a2, b2)] = idx
                        idx += 1

                def g_tile(a2, b2):
                    if b2 >= a2:
                        return gsb[:, a2, 128 * b2:128 * (b2 + 1)]
                    return gtr[:, low[(a2, b2)], :]

                # A = Gc @ Wv  (A[c, e]), bf16
                wh = whi[s]
                ab = ap_.tile([128, CB, C], BF, tag="ab", name=f"ab{s}")
                for b2 in range(CB):
                    apx = psa.tile([128, C], F32, tag=("apx", "ctp")[b2 % 2],
                                   name=f"apx{s}_{b2}")
                    for a2 in range(CB):
                        nc.tensor.matmul(
                            apx[:], lhsT=g_tile(a2, b2), rhs=wh[:, a2, C:],
                            start=(a2 == 0), stop=(a2 == CB - 1))
                    copy_alt(b2, ab[:, b2, :], apx[:])

                # ctp[e, h, d] = A^T Wk per head (PSUM fp32), in two
                # head-group halves so softmax can start on half 0 while
                # half 1 is still on the PE.
                ctps = []
                for g in range(2):
                    ctp = psa.tile([HD, 4, HD], F32, tag=("ctp", "apx")[g],
                                   name=f"ctp{s}_{g}")
                    for hh in range(4):
                        h = 4 * g + hh
                        sl = slice(HD * h, HD * (h + 1))
                        for a2 in range(CB):
                            nc.tensor.matmul(
                                ctp[:, hh, :], lhsT=ab[:, a2, sl],
                                rhs=wh[:, a2, sl],
                                start=(a2 == 0), stop=(a2 == CB - 1))
                    ctps.append(ctp)
                return ctps

            def ctx_softmax(s, ctps):
                """Stable softmax over d (ACT/DVE only), per head-group."""
                cts = []
                for g in range(2):
                    comb = cxp.tile([HD, 4, HD], F32, tag=f"comb{g}",
                                    name=f"comb{s}_{g}")
                    nc.vector.tensor_add(comb[:], ctps[g][:],
                                         tts[s][:, 4 * g:4 * (g + 1), :])
                    mx = cxp.tile([HD, 4], F32, tag=f"mx{g}", name=f"mx{s}_{g}")
                    nc.vector.tensor_reduce(
                        mx[:], comb[:], axis=mybir.AxisListType.X,
                        op=mybir.AluOpType.max)
                    nbias = cxp.tile([HD, 4], F32, tag=f"nbias{g}",
                                     name=f"nb{s}_{g}")
                    nc.scalar.mul(nbias[:], mx[:], -SCALE)
                    esb = cxp.tile([HD, 4, HD], F32, tag=f"esb{g}",
                                   name=f"esb{s}_{g}")
                    ssum = cxp.tile([HD, 4], F32, tag=f"ssum{g}",
                                    name=f"ss{s}_{g}")
                    rsum = cxp.tile([HD, 4], F32, tag=f"rsum{g}",
                                    name=f"rs{s}_{g}")
                    ctxts = cxp.tile([HD, 4, HD], BF, tag=f"ctxts{g}",
                                     name=f"ct{s}_{g}")
                    for hh in range(4):
                        nc.scalar.activation(
                            esb[:, hh, :], comb[:, hh, :], AF.Exp, scale=SCALE,
                            bias=nbias[:, hh:hh + 1],
                            accum_out=ssum[:, hh:hh + 1])
                    nc.vector.reciprocal(rsum[:], ssum[:])
                    for hh in range(4):
                        nc.vector.tensor_scalar_mul(
                            ctxts[:, hh, :], esb[:, hh, :], rsum[:, hh:hh + 1])
                    cts.append(ctxts)
                return cts

            def ctx_cbd(s, cts):
                """Per-head transpose of ctx into block-diag cbd tiles."""
                for cb in range(CB):
                    nc.gpsimd.memset(cbds[s][cb][:], 0.0)
                for h in range(H):
                    tp = pst.tile([128, 128], BF, tag="tps", name=f"cb{s}_{h}")
                    t2, r2 = h // 2, (h % 2) * HD
                    nc.tensor.transpose(
                        tp[r2:r2 + HD, r2:r2 + HD],
                        cts[h // 4][:, h % 4, :],
                        ident[0:HD, 0:HD], tile_position=(0, r2))
                    copy_alt(h, cbds[s][t2][r2:r2 + HD, r2:r2 + HD],
                             tp[r2:r2 + HD, r2:r2 + HD])

            # ---------------- phase program ----------------
            def gram_psum(s):
                gps = []
                for m in range(CB):
                    gt_ = psg.tile([128, C - 128 * m], F32, tag=f"gp{m}",
                                   name=f"gp{m}_{s}")
                    gps.append(gt_[:])
                return gps

            # phase 0: stream x1 -> G1, xt1. x chunk loads are issued
            # before the W/TT loads so the PE isn't gated on them.
            gps0 = gram_psum(0)
            xc0s = {r: load_chunk(0, r) for r in range(3)}

            whi, tts = [], []
            for s in range(2):
                wh = wp.tile([128, CB, 2 * C], BF, tag=f"w{s}", name=f"w{s}")
                nc.sync.dma_start(
                    out=wh[:],
                    in_=w_d[s][:, :].rearrange("(a p) m -> p a m", p=128))
                whi.append(wh)
                tt = constp.tile([HD, H, HD], F32, tag=f"tt{s}", name=f"tt{s}")
                nc.sync.dma_start(out=tt[:], in_=t_d[s][:, :])
                tts.append(tt)

            for r in range(NCH):
                xc = xc0s[r] if r in xc0s else load_chunk(0, r)
                for tt_ in range(4):
                    gram_tile(0, gps0, xc, tt_, 4 * r + tt_)

            # ctx1 part A (PE-dense), then prefetch x2 and start gram(1)
            # so the softmax(0) chain overlaps with gram compute.
            ctp0 = ctx_a(0, gps0)
            gps1 = gram_psum(1)
            xc1s = {r: load_chunk(1, r) for r in range(4)}
            for r in range(2):
                for tt_ in range(4):
                    gram_tile(1, gps1, xc1s[r], tt_, 4 * r + tt_)
            ctxts0 = ctx_softmax(0, ctp0)
            ctx_cbd(0, ctxts0)

            # rest of gram(1) fused with most of o2 = x2 @ cbd1 (lag 2);
            # the last 8 o2 tiles are held back to cover the softmax(1)
            # window after ctx2 part A.
            HOLD = NT - 16
            done_t = 0
            for r in range(2, NCH):
                xc = xc1s[r] if r in xc1s else load_chunk(1, r)
                for tt_ in range(4):
                    t = 4 * r + tt_
                    gram_tile(1, gps1, xc, tt_, t)
                    while done_t <= t - 2 and done_t < HOLD:
                        out_tile(1, done_t)
                        done_t += 1

            ctp1 = ctx_a(1, gps1)
            ctxts1 = ctx_softmax(1, ctp1)
            while done_t < NT:
                out_tile(1, done_t, gp_tags=True)
                done_t += 1
            ctx_cbd(1, ctxts1)

            # o1 = x1 @ cbd2 from stored xt1 (4 PSUM slots via gram banks)
            for t in range(NT):
                out_tile(0, t, gp_tags=True)

    nc.compile()
    return nc


_NC = None


def _host_tt(W):
    """mu * Wv_h^T Wk_h packed [HD, H*HD] fp32, exact in fp64."""
    W64 = np.asarray(W, dtype=np.float64)
    out = np.empty((HD, C), dtype=np.float32)
    for h in range(H):
        wv = W64[:, C + HD * h: C + HD * (h + 1)]
        wk = W64[:, HD * h: HD * (h + 1)]
        out[:, HD * h: HD * (h + 1)] = (MU * (wv.T @ wk)).astype(np.float32)
    return out


def make_in_map(inputs, b):
    bfc = lambda a: np.ascontiguousarray(a).astype(ml_dtypes.bfloat16)
    return {
        "x1": bfc(inputs["x1"][b]),
        "x2": bfc(inputs["x2"][b]),
        "w1": bfc(inputs["W_kv1"]),
        "w2": bfc(inputs["W_kv2"]),
        "tt1": _host_tt(inputs["W_kv1"]),
        "tt2": _host_tt(inputs["W_kv2"]),
    }


def postprocess(results):
    o1 = np.stack([results[b]["o1"].astype(np.float32) for b in range(B)])
    o2 = np.stack([results[b]["o2"].astype(np.float32) for b in range(B)])
    return o1, o2


def kernel(x1, x2, W_kv1, W_kv2):
    global _NC
    if _NC is None:
        _NC = build()
    inputs = {"x1": x1, "x2": x2, "W_kv1": W_kv1, "W_kv2": W_kv2}
    in_maps = [make_in_map(inputs, b) for b in range(B)]
    res = run_bass_kernel_spmd(_NC, in_maps, core_ids=list(range(B)))
    return postprocess(res.results)


# revision 27
# speedup vs baseline: 1.0528x; 1.0528x over previous
"""Trainium2 Bass kernel for nn_CrossAttention_249108103802.

Math (per batch b, one NeuronCore; 8 cores data-parallel over B=8):
  q_s   = heads(x_s)                   (column slices of x_s)
  k,v   = x_s @ W_s  split per head    -> never materialized; instead
  ctx_s = softmax_d(scale * k^T v)     via the Gram trick:
          k_h^T v_h = Wk_h^T (x^T x) Wv_h
  o1    = q1 @ blockdiag(ctx2), o2 = q2 @ blockdiag(ctx1)

Precision: x and W are cast to bf16 on the host; all PE matmuls run in
bf16 with fp32 PSUM accumulation. The Gram matrix is split G = Gc + mu*I
(mu = N) so Gc fits bf16; the exact mu * Wv^T Wk correction is computed
on the host in fp64 and shipped as an fp32 input. Softmax subtracts the
per-row max before exp (the logits reach ~92, which overflows fp32 exp).
Measured end-to-end rel err ~4e-3 vs the fp32 reference.
"""
import sys

sys.path.insert(0, "/opt/trn_rl_repo")

import ml_dtypes
import numpy as np

import concourse.bass as bass
import concourse.mybir as mybir
import concourse.tile as tile
from concourse import bacc
from concourse.bass_utils import run_bass_kernel_spmd
from concourse.masks import make_identity

B, N, C, H = 8, 4096, 512, 8
HD = C // H                    # 64
SCALE = HD ** -0.5             # 1/8
MU = float(N)                  # expected Gram diagonal
NT = N // 128                  # 32 row tiles
CB = C // 128                  # 4 feature blocks
NCH = NT // 4                  # 8 chunks of 4 row tiles
BF = mybir.dt.bfloat16
F32 = mybir.dt.float32
AF = mybir.ActivationFunctionType


def build():
    nc = bacc.Bacc("TRN2", target_bir_lowering=False, debug=False, num_devices=8)
    x_d = [nc.declare_dram_parameter("x1", [N, C], BF, isOutput=False),
           nc.declare_dram_parameter("x2", [N, C], BF, isOutput=False)]
    w_d = [nc.declare_dram_parameter("w1", [C, 2 * C], BF, isOutput=False),
           nc.declare_dram_parameter("w2", [C, 2 * C], BF, isOutput=False)]
    t_d = [nc.declare_dram_parameter("tt1", [HD, C], F32, isOutput=False),
           nc.declare_dram_parameter("tt2", [HD, C], F32, isOutput=False)]
    o_d = [nc.declare_dram_parameter("o1", [N, C], BF, isOutput=True),
           nc.declare_dram_parameter("o2", [N, C], BF, isOutput=True)]

    with tile.TileContext(nc) as tc:
        with (
            tc.tile_pool(name="const", bufs=1) as constp,
            tc.tile_pool(name="w", bufs=1) as wp,
            tc.tile_pool(name="x0", bufs=4) as xp0,
            tc.tile_pool(name="x1", bufs=4) as xp1,
            tc.tile_pool(name="xt", bufs=1) as xtp,
            tc.tile_pool(name="g", bufs=1) as gp_,
            tc.tile_pool(name="a", bufs=1) as ap_,
            tc.tile_pool(name="ctx", bufs=1) as cxp,
            tc.tile_pool(name="osb", bufs=3) as osp,
            tc.tile_pool(name="ps_g", bufs=1, space="PSUM") as psg,
            tc.tile_pool(name="ps_t", bufs=2, space="PSUM") as pst,
            tc.tile_pool(name="ps_a", bufs=1, space="PSUM") as psa,
        ):
            ident = constp.tile([128, 128], BF, tag="ident")
            make_identity(nc, ident[:])
            muI = constp.tile([128, 128], F32, tag="muI")
            nc.gpsimd.memset(muI[:], 0.0)
            nc.gpsimd.affine_select(
                out=muI[:], in_=muI[:],
                compare_op=mybir.AluOpType.not_equal, fill=MU,
                base=0, pattern=[[-1, 128]], channel_multiplier=1,
            )

            def copy_alt(i, out, in_):
                if i % 2 == 0:
                    nc.scalar.copy(out, in_)
                else:
                    nc.vector.tensor_copy(out, in_)

            xts, cbds = [], []
            for s in range(2):
                xts.append(xtp.tile([128, CB, N], BF, tag=f"xt{s}",
                                    name=f"xt{s}"))
                cbds.append([cxp.tile([128, 128], BF, tag=f"cbd{s}_{cb}",
                                      name=f"cbd{s}_{cb}") for cb in range(CB)])

            def load_chunk(s, r):
                """DMA one [512, C] chunk of x_s as bf16 into SBUF."""
                xp = xp0 if s == 0 else xp1
                xc = xp.tile([128, 4, C], BF, tag=f"xc{s}", name=f"xc{s}_{r}")
                src = x_d[s][512 * r:512 * (r + 1), :].rearrange(
                    "(t p) c -> p t c", p=128)
                nc.sync.dma_start(out=xc[:], in_=src)
                return xc

            tp8s = {}

            def gram_tile(s, gps, xc, tt_, t):
                """Gram accum + transpose of one [128, C] row tile.
                Transposes of tile pairs (2t, 2t+1) land in one PSUM bank,
                laid out [cb][t%2][col], and are evacuated with one copy."""
                for m in range(CB):
                    nc.tensor.matmul(
                        gps[m],
                        lhsT=xc[:, tt_, 128 * m:128 * (m + 1)],
                        rhs=xc[:, tt_, 128 * m:],
                        start=(t == 0), stop=(t == NT - 1),
                    )
                if t % 2 == 0:
                    tp8s[s] = pst.tile([128, CB, 2, 128], BF, tag="tps",
                                       name=f"tp8_{s}_{t}")
                tp8 = tp8s[s]
                for cb in range(CB):
                    nc.tensor.transpose(
                        tp8[:, cb, t % 2, :], xc[:, tt_, 128 * cb:128 * (cb + 1)],
                        ident[:])
                if t % 2 == 1:
                    copy_alt(t // 2, xts[s][:, :, 128 * (t - 1):128 * (t + 1)],
                             tp8[:])

            obs = {}

            def out_tile(s, t, gp_tags=False):
                """One [128, C] row tile of o_s = x_s @ blockdiag(ctx_other):
                matmul into a rotating PSUM slot, then copy into the chunk's
                output staging tile; DMA the chunk once its 4 tiles landed.
                gp_tags=True cycles through the idle Gram banks (4 slots)
                instead of the 2 ctx banks."""
                r, tt_ = t // 4, t % 4
                if tt_ == 0:
                    obs[(s, r)] = osp.tile([128, 4, C], BF, tag="ob",
                                           name=f"ob{s}_{r}")
                if gp_tags:
                    op = psg.tile([128, C], F32, tag=f"gp{t % 4}",
                                  name=f"op{s}_{t}")
                else:
                    op = psa.tile([128, C], F32, tag=("apx", "ctp")[t % 2],
                                  name=f"op{s}_{t}")
                for cb in range(CB):
                    nc.tensor.matmul(
                        op[:, 128 * cb:128 * (cb + 1)],
                        lhsT=xts[s][:, cb, 128 * t:128 * (t + 1)],
                        rhs=cbds[1 - s][cb][:, :],
                        start=True, stop=True)
                ob = obs[(s, r)]
                copy_alt(t, ob[:, tt_, :], op[:])
                if tt_ == 3:
                    dst = o_d[s][512 * r:512 * (r + 1), :].rearrange(
                        "(t p) c -> p t c", p=128)
                    nc.sync.dma_start(out=dst, in_=ob[:])

            def ctx_a(s, gps):
                """PE-heavy first half: G_s -> Gc -> A -> ctp logits (PSUM)."""
                # Gc = G - mu*I in bf16; gsb[:, a, f] = G[128a+p, f]
                gsb = gp_.tile([128, CB, C], BF, tag="gsb", name=f"g{s}")
                for m in range(CB):
                    nc.vector.tensor_sub(
                        gsb[:, m, 128 * m:128 * (m + 1)],
                        gps[m][:, 0:128], muI[:])
                    if m < CB - 1:
                        copy_alt(m, gsb[:, m, 128 * (m + 1):], gps[m][:, 128:])
                # lower-triangle tiles by PE transpose
                low = {}
                gtr = gp_.tile([128, 6, 128], BF, tag="gtr", name=f"gt{s}")
                idx = 0
                for a2 in range(CB):
                    for b2 in range(a2):
                        tp = pst.tile([128, 128], BF, tag="tps",
                                      name=f"gtp{s}_{idx}")
                        nc.tensor.transpose(
                            tp[:], gsb[:, b2, 128 * a2:128 * (a2 + 1)], ident[:])
                        copy_alt(idx, gtr[:, idx, :], tp[:])
                        low[(a2, b2)] = idx
                        idx += 1

                def g_tile(a2, b2):
                    if b2 >= a2:
                        return gsb[:, a2, 128 * b2:128 * (b2 + 1)]
                    return gtr[:, low[(a2, b2)], :]

                # A = Gc @ Wv  (A[c, e]), bf16
                wh = whi[s]
                ab = ap_.tile([128, CB, C], BF, tag="ab", name=f"ab{s}")
                for b2 in range(CB):
                    apx = psa.tile([128, C], F32, tag=("apx", "ctp")[b2 % 2],
                                   name=f"apx{s}_{b2}")
                    for a2 in range(CB):
                        nc.tensor.matmul(
                            apx[:], lhsT=g_tile(a2, b2), rhs=wh[:, a2, C:],
                            start=(a2 == 0), stop=(a2 == CB - 1))
                    copy_alt(b2, ab[:, b2, :], apx[:])

                # ctp[e, h, d] = A^T Wk per head (PSUM fp32)
                ctp = psa.tile([HD, H, HD], F32, tag="ctp", name=f"ctp{s}")
                for h in range(H):
                    sl = slice(HD * h, HD * (h + 1))
                    for a2 in range(CB):
                        nc.tensor.matmul(
                            ctp[:, h, :], lhsT=ab[:, a2, sl], rhs=wh[:, a2, sl],
                            start=(a2 == 0), stop=(a2 == CB - 1))
                return ctp

            def ctx_softmax(s, ctp):
                """Stable softmax over d (ACT/DVE only)."""
                comb = cxp.tile([HD, H, HD], F32, tag="comb", name=f"comb{s}")
                nc.vector.tensor_add(comb[:], ctp[:], tts[s][:])
                mx = cxp.tile([HD, H], F32, tag="mx", name=f"mx{s}")
                nc.vector.tensor_reduce(
                    mx[:], comb[:], axis=mybir.AxisListType.X,
                    op=mybir.AluOpType.max)
                nbias = cxp.tile([HD, H], F32, tag="nbias", name=f"nb{s}")
                nc.scalar.mul(nbias[:], mx[:], -SCALE)
                esb = cxp.tile([HD, H, HD], F32, tag="esb", name=f"esb{s}")
                ssum = cxp.tile([HD, H], F32, tag="ssum", name=f"ss{s}")
                rsum = cxp.tile([HD, H], F32, tag="rsum", name=f"rs{s}")
                ctxts = cxp.tile([HD, H, HD], BF, tag="ctxts", name=f"ct{s}")
                for h in range(H):
                    nc.scalar.activation(
                        esb[:, h, :], comb[:, h, :], AF.Exp, scale=SCALE,
                        bias=nbias[:, h:h + 1], accum_out=ssum[:, h:h + 1])
                nc.vector.reciprocal(rsum[:], ssum[:])
                for h in range(H):
                    nc.vector.tensor_scalar_mul(
                        ctxts[:, h, :], esb[:, h, :], rsum[:, h:h + 1])
                return ctxts

            def ctx_cbd(s, ctxts):
                """Per-head transpose of ctx into block-diag cbd tiles."""
                for cb in range(CB):
                    nc.gpsimd.memset(cbds[s][cb][:], 0.0)
                for h in range(H):
                    tp = pst.tile([128, 128], BF, tag="tps", name=f"cb{s}_{h}")
                    t2, r2 = h // 2, (h % 2) * HD
                    nc.tensor.transpose(
                        tp[r2:r2 + HD, r2:r2 + HD],
                        ctxts[:, h, :],
                        ident[0:HD, 0:HD], tile_position=(0, r2))
                    copy_alt(h, cbds[s][t2][r2:r2 + HD, r2:r2 + HD],
                             tp[r2:r2 + HD, r2:r2 + HD])

            # ---------------- phase program ----------------
            def gram_psum(s):
                gps = []
                for m in range(CB):
                    gt_ = psg.tile([128, C - 128 * m], F32, tag=f"gp{m}",
                                   name=f"gp{m}_{s}")
                    gps.append(gt_[:])
                return gps

            # phase 0: stream x1 -> G1, xt1. x chunk loads are issued
            # before the W/TT loads so the PE isn't gated on them.
            gps0 = gram_psum(0)
            xc0s = {r: load_chunk(0, r) for r in range(3)}

            whi, tts = [], []
            for s in range(2):
                wh = wp.tile([128, CB, 2 * C], BF, tag=f"w{s}", name=f"w{s}")
                nc.sync.dma_start(
                    out=wh[:],
                    in_=w_d[s][:, :].rearrange("(a p) m -> p a m", p=128))
                whi.append(wh)
                tt = constp.tile([HD, H, HD], F32, tag=f"tt{s}", name=f"tt{s}")
                nc.sync.dma_start(out=tt[:], in_=t_d[s][:, :])
                tts.append(tt)

            for r in range(NCH):
                xc = xc0s[r] if r in xc0s else load_chunk(0, r)
                for tt_ in range(4):
                    gram_tile(0, gps0, xc, tt_, 4 * r + tt_)

            # ctx1 part A (PE-dense), then prefetch x2 and start gram(1)
            # so the softmax(0) chain overlaps with gram compute.
            ctp0 = ctx_a(0, gps0)
            gps1 = gram_psum(1)
            xc1s = {r: load_chunk(1, r) for r in range(4)}
            for r in range(2):
                for tt_ in range(4):
                    gram_tile(1, gps1, xc1s[r], tt_, 4 * r + tt_)
            ctxts0 = ctx_softmax(0, ctp0)
            ctx_cbd(0, ctxts0)

            # rest of gram(1) fused with most of o2 = x2 @ cbd1 (lag 2);
            # the last 8 o2 tiles are held back to cover the softmax(1)
            # window after ctx2 part A.
            HOLD = NT - 16
            done_t = 0
            for r in range(2, NCH):
                xc = xc1s[r] if r in xc1s else load_chunk(1, r)
                for tt_ in range(4):
                    t = 4 * r + tt_
                    gram_tile(1, gps1, xc, tt_, t)
                    while done_t <= t - 2 and done_t < HOLD:
                        out_tile(1, done_t)
                        done_t += 1

            ctp1 = ctx_a(1, gps1)
            ctxts1 = ctx_softmax(1, ctp1)
            while done_t < NT:
                out_tile(1, done_t, gp_tags=True)
                done_t += 1
            ctx_cbd(1, ctxts1)

            # o1 = x1 @ cbd2 from stored xt1 (4 PSUM slots via gram banks)
            for t in range(NT):
                out_tile(0, t, gp_tags=True)

    nc.compile()
    return nc


_NC = None


def _host_tt(W):
    """mu * Wv_h^T Wk_h packed [HD, H*HD] fp32, exact in fp64."""
    W64 = np.asarray(W, dtype=np.float64)
    out = np.empty((HD, C), dtype=np.float32)
    for h in range(H):
        wv = W64[:, C + HD * h: C + HD * (h + 1)]
        wk = W64[:, HD * h: HD * (h + 1)]
        out[:, HD * h: HD * (h + 1)] = (MU * (wv.T @ wk)).astype(np.float32)
    return out


def make_in_map(inputs, b):
    bfc = lambda a: np.ascontiguousarray(a).astype(ml_dtypes.bfloat16)
    return {
        "x1": bfc(inputs["x1"][b]),
        "x2": bfc(inputs["x2"][b]),
        "w1": bfc(inputs["W_kv1"]),
        "w2": bfc(inputs["W_kv2"]),
        "tt1": _host_tt(inputs["W_kv1"]),
        "tt2": _host_tt(inputs["W_kv2"]),
    }


def postprocess(results):
    o1 = np.stack([results[b]["o1"].astype(np.float32) for b in range(B)])
    o2 = np.stack([results[b]["o2"].astype(np.float32) for b in range(B)])
    return o1, o2


def kernel(x1, x2, W_kv1, W_kv2):
    global _NC
    if _NC is None:
        _NC = build()
    inputs = {"x1": x1, "x2": x2, "W_kv1": W_kv1, "W_kv2": W_kv2}
    in_maps = [make_in_map(inputs, b) for b in range(B)]
    res = run_bass_kernel_spmd(_NC, in_maps, core_ids=list(range(B)))
    return postprocess(res.results)
